# revision 29
# baseline (speedup 1.0000x reference)
"""Trainium2 Bass kernel for nn_Attention2d (N=32, C=128, S=32*36=1152, OUT=5000).

Math (per image i):
    xe = x.reshape(C,S) + pos                      # (C,S)
    scores[s,n] = sum_c xe[c,s] * nq[c,n]          # QK, contraction over C
    attn = softmax_s(scores)
    y[n] = sum_c f[c,n] * sum_s x[c,s]*attn[s,n] + bias[n]

Reformulation used here (all big matmuls contract over C=128 = partition dim):
    Z[s,n]   = sum_c x[c,s] * f[c,n]               # same shape/layout as scores
    E        = exp(scores)                          # no max-subtraction needed
    num[n]   = sum_s E[s,n] * Z[s,n]
    den[n]   = sum_s E[s,n]
    y[n]     = num[n]/den[n] + bias[n]

Layout: transposed [n_partition, s_free] tiles so that:
  - scoresT/ZT chunks come from matmuls lhsT=nq/f[:, nchunk(128)], rhs=xe/x[:, s]
  - exp runs on ACT with accum_out -> den  (free-dim = s reduction)
  - num comes from fused DVE passes: scalar_tensor_tensor(E * ZT, accum=sum_s)
    (TensorTensorReduce crashes the DVE on this silicon; STT's accum works.)

Pipeline: the 8 PSUM banks are the scarce resource. scoresT tiles [128,1152]
(3 banks) are double-buffered (6 banks) so the next image's QK matmuls overlap
the current exp; ZT streams through a 2-slot ring of 1-bank [128,512] tiles,
with the fused multiply split into 3 sub-passes (512/512/128) whose partial
sums land in separate accumulator columns merged at image end. The loop is
chunk-outer/image-inner (weight loads amortize over 4 images) and Z/STT work
is emitted with a 1-image lag so the in-order PE alternates S-bursts with Z
pieces. Inputs are cast to bf16 on the host (halves DMA, enables 1-cyc/row
matmuls); startup DMAs are split and spread across engine queues; per-image
epilogues run pipelined with the last chunk's compute. Z/STT pieces are a
uniform 384 wide: with 512/512/128 pieces every 512-piece waited ~240ns for
its just-in-time Z matmul (ring depth 2 is bank-limited); uniform pieces
equalize the DVE inter-piece window with the Z-matmul chain and put the DVE
at ~96% busy. Measured: 286-288us across runs, rel err 2.87e-3 (bf16
matmul class).

Sharding: batch N=32 across 8 cores (4 images/core), no collectives.
"""

import os
import sys

for _p in ("/opt/trn_rl_repo", "/root/.axon_site/_ro/trn_rl_repo"):
    if os.path.isdir(_p) and _p not in sys.path:
        sys.path.append(_p)

import ml_dtypes
import numpy as np

BF16 = ml_dtypes.bfloat16

N, C, W, H = 32, 128, 32, 36
S = W * H          # 1152
OUT = 5000
CORES = 8
IPC = N // CORES   # images per core = 4
NCH = 40           # n-chunks of 128 partitions (OUT padded to 5120)
OUTP = NCH * 128   # 5120

S_SLICES = [(0, 512), (512, 1024), (1024, 1152)]
# uniform Z/STT pieces: equalizes the DVE inter-piece window with the
# just-in-time Z-matmul chain (512/512/128 made every 512-piece wait ~240ns)
Z_SLICES = [(0, 384), (384, 768), (768, 1152)]

_CACHE = {}


def _build_nc():
    import concourse.tile as tile
    from concourse import bacc, mybir
    from concourse.masks import make_identity

    f32 = mybir.dt.float32
    bf16 = mybir.dt.bfloat16
    EXP = mybir.ActivationFunctionType.Exp
    MULT = mybir.AluOpType.mult
    ADD = mybir.AluOpType.add
    X = mybir.AxisListType.X

    nc = bacc.Bacc()

    x_d = nc.dram_tensor("x", [IPC + 1, C, S], bf16, kind="ExternalInput")
    nq_d = nc.dram_tensor("neuron_query", [C, OUT], bf16, kind="ExternalInput")
    f_d = nc.dram_tensor("features", [C, OUT], bf16, kind="ExternalInput")
    b_d = nc.dram_tensor("bias", [OUT], f32, kind="ExternalInput")
    o_d = nc.dram_tensor("out", [IPC, OUT], f32, kind="ExternalOutput")

    with tile.TileContext(nc) as tc:
        with (
            tc.tile_pool(name="singles", bufs=1) as singles,
            tc.tile_pool(name="imgs", bufs=IPC) as imgs_pool,
            tc.tile_pool(name="accs", bufs=2 * IPC) as acc_pool,
            tc.tile_pool(name="epool", bufs=6) as e_pool,
            tc.tile_pool(name="scpool", bufs=2) as sc_pool,
            tc.tile_pool(name="epi", bufs=2) as epi_pool,
            tc.tile_pool(name="psS", bufs=2, space="PSUM") as psS,
            tc.tile_pool(name="psZ", bufs=2, space="PSUM") as psZ,
        ):
            # ---- one-time loads ----
            # Queue order tuned so chunk 0's deps land first: sync carries
            # pos, x0 and a small first weight piece; gpsimd carries f
            # pieces; scalar carries the remaining images.
            # x0 and pos (packed by the host as x slice IPC) arrive in ONE
            # fused DMA - one less issue+transfer on the startup chain
            x0p_t = singles.tile([C, 2, S], bf16, tag="x0p")
            nc.sync.dma_start(
                out=x0p_t,
                in_=x_d[0 : IPC + 1 : IPC, :, :].rearrange("o p s -> p o s"),
            )
            pos_t = x0p_t[:, 1, :]

            xe_l, xf_l, num3_l, den_l = [], [], [], []
            for i in range(IPC):
                if i == 0:
                    xf_mm = x0p_t[:, 0, :]
                else:
                    xf_mm = imgs_pool.tile([C, S], bf16, tag="xfb")
                    nc.scalar.dma_start(out=xf_mm, in_=x_d[i])
                xe_mm = imgs_pool.tile([C, S], bf16, tag="xe")
                nc.vector.tensor_add(xe_mm, xf_mm, pos_t)  # bf16 2x mode
                xe_l.append(xe_mm)
                xf_l.append(xf_mm)
                num3_t = acc_pool.tile([128, NCH * 3], f32, tag="num3")
                den_t = acc_pool.tile([128, NCH], f32, tag="den")
                num3_l.append(num3_t)
                den_l.append(den_t)

            # weight tiles in pieces (small first piece) so the first
            # chunks' matmuls start after a fraction of the DMA
            PIECES = [512, 1536, 1536, 1536]  # cols per piece, sum=OUTP
            nq_tiles, f_tiles = [], []
            piece_of = []  # chunk -> (piece idx, col offset)
            lo = 0
            for pi, w in enumerate(PIECES):
                hi_real = min(OUT, lo + w)
                nq_p = singles.tile([C, w], bf16, tag=f"nq{pi}")
                f_p = singles.tile([C, w], bf16, tag=f"f{pi}")
                for dram, dst, eng in (
                    (nq_d, nq_p, nc.sync),
                    (f_d, f_p, nc.gpsimd),
                ):
                    if hi_real < lo + w:
                        nc.gpsimd.memset(dst[:, hi_real - lo : w], 0.0)
                    if hi_real > lo:
                        eng.dma_start(
                            out=dst[:, 0 : hi_real - lo], in_=dram[:, lo:hi_real]
                        )
                nq_tiles.append(nq_p)
                f_tiles.append(f_p)
                for c in range(lo // 128, (lo + w) // 128):
                    piece_of.append((pi, c * 128 - lo))
                lo += w

            ident = singles.tile([128, 128], f32)
            make_identity(nc, ident)

            bias_t = singles.tile([128, NCH], f32)
            nc.gpsimd.memset(bias_t, 0.0)
            nc.sync.dma_start(
                out=bias_t[:, 0:39],
                in_=b_d[0 : 39 * 128].rearrange("(c p) -> p c", p=128),
            )
            nc.sync.dma_start(
                out=bias_t[0:8, 39:40],
                in_=b_d[39 * 128 : OUT].rearrange("(p o) -> p o", o=1),
            )

            # ---- main loop: chunk-outer, image-inner ----
            # Z/STT pieces for each image are emitted with a 1-image lag so
            # the in-order PE interleaves S-bursts with Z pieces and each
            # STT's inputs (exp output + Z matmul) are ready when it issues.
            def emit_epilogue(i):
                num_t = epi_pool.tile([128, NCH], f32, tag="num")
                nc.vector.tensor_reduce(
                    out=num_t,
                    in_=num3_l[i].rearrange("p (c j) -> p c j", j=3),
                    op=ADD,
                    axis=X,
                )
                rcp_t = epi_pool.tile([128, NCH], f32, tag="rcp")
                nc.vector.reciprocal(out=rcp_t, in_=den_l[i])
                y1_t = epi_pool.tile([128, NCH], f32, tag="y1")
                nc.vector.tensor_mul(y1_t, num_t, rcp_t)
                y_t = epi_pool.tile([128, NCH], f32, tag="y")
                nc.vector.tensor_add(y_t, y1_t, bias_t)

                pt = psZ.tile([NCH, 128], f32, tag="z")
                nc.tensor.transpose(out=pt, in_=y_t, identity=ident)
                yT_t = epi_pool.tile([NCH, 128], f32, tag="yT")
                nc.scalar.copy(out=yT_t, in_=pt)

                nc.sync.dma_start(
                    out=o_d[i, 0 : 39 * 128].rearrange("(c p) -> c p", p=128),
                    in_=yT_t[0:39, :],
                )
                nc.sync.dma_start(
                    out=o_d[i, 39 * 128 : OUT].rearrange("(o p) -> o p", o=1),
                    in_=yT_t[39:40, 0:8],
                )

            pending = None  # (e_t, img, chunk, f_c)

            def emit_z(e_t, i, c, f_c):
                for j, (lo, hi) in enumerate(Z_SLICES):
                    w = hi - lo
                    z_t = psZ.tile([128, 512], f32, tag="z")
                    nc.tensor.matmul(
                        z_t[:, 0:w], f_c, xf_l[i][:, lo:hi],
                        start=True, stop=True,
                    )
                    sc_t = sc_pool.tile([128, 512], f32, tag="sc")
                    col = c * 3 + j
                    nc.vector.scalar_tensor_tensor(
                        out=sc_t[:, 0:w],
                        in0=e_t[:, lo:hi],
                        scalar=1.0,
                        in1=z_t[:, 0:w],
                        op0=MULT,
                        op1=MULT,
                        accum_out=num3_l[i][:, col : col + 1],
                    )

            for c in range(NCH):
                pi, po = piece_of[c]
                nq_c = nq_tiles[pi][:, po : po + 128]
                f_c = f_tiles[pi][:, po : po + 128]
                for i in range(IPC):
                    s_t = psS.tile([128, S], f32, tag="s")
                    for lo, hi in S_SLICES:
                        nc.tensor.matmul(
                            s_t[:, lo:hi], nq_c, xe_l[i][:, lo:hi],
                            start=True, stop=True,
                        )
                    e_t = e_pool.tile([128, S], f32, tag="e")
                    nc.scalar.activation(
                        out=e_t, in_=s_t, func=EXP,
                        accum_out=den_l[i][:, c : c + 1],
                    )
                    if pending is not None:
                        emit_z(*pending)
                        if pending[2] == NCH - 1:
                            emit_epilogue(pending[1])
                    pending = (e_t, i, c, f_c)
            emit_z(*pending)
            emit_epilogue(pending[1])

    nc.compile()
    return nc


def _get_nc():
    if "nc" not in _CACHE:
        _CACHE["nc"] = _build_nc()
    return _CACHE["nc"]


def _prep_in_maps(inputs):
    xf = np.ascontiguousarray(
        np.asarray(inputs["x"], dtype=np.float32).reshape(N, C, S).astype(BF16)
    )
    nq = np.ascontiguousarray(
        np.asarray(inputs["neuron_query"], dtype=np.float32)
        .reshape(C, OUT)
        .astype(BF16)
    )
    ft = np.ascontiguousarray(
        np.asarray(inputs["features"], dtype=np.float32)
        .reshape(C, OUT)
        .astype(BF16)
    )
    pos = np.ascontiguousarray(
        np.asarray(inputs["pos_emb"], dtype=np.float32).reshape(C, S).astype(BF16)
    )
    bias = np.ascontiguousarray(np.asarray(inputs["bias"], dtype=np.float32))
    return [
        {
            "x": np.ascontiguousarray(
                np.concatenate([xf[i * IPC : (i + 1) * IPC], pos[None]], axis=0)
            ),
            "neuron_query": nq,
            "features": ft,
            "bias": bias,
        }
        for i in range(CORES)
    ]


def run_kernel(inputs, trace=False):
    """Returns (out [N, OUT] float32, BassKernelResults)."""
    from concourse.bass_utils import run_bass_kernel_spmd

    nc = _get_nc()
    in_maps = _prep_in_maps(inputs)
    res = run_bass_kernel_spmd(nc, in_maps, list(range(CORES)), trace=trace)
    out = np.concatenate([r["out"] for r in res.results], axis=0)
    return np.asarray(out, dtype=np.float32), res


def kernel(**inputs):
    out, _ = run_kernel(inputs, trace=False)
    return out


# revision 30
# speedup vs baseline: 1.0021x; 1.0021x over previous
"""Trainium2 Bass kernel for nn_Attention2d (N=32, C=128, S=32*36=1152, OUT=5000).

Math (per image i):
    xe = x.reshape(C,S) + pos                      # (C,S)
    scores[s,n] = sum_c xe[c,s] * nq[c,n]          # QK, contraction over C
    attn = softmax_s(scores)
    y[n] = sum_c f[c,n] * sum_s x[c,s]*attn[s,n] + bias[n]

Reformulation used here (all big matmuls contract over C=128 = partition dim):
    Z[s,n]   = sum_c x[c,s] * f[c,n]               # same shape/layout as scores
    E        = exp(scores)                          # no max-subtraction needed
    num[n]   = sum_s E[s,n] * Z[s,n]
    den[n]   = sum_s E[s,n]
    y[n]     = num[n]/den[n] + bias[n]

Layout: transposed [n_partition, s_free] tiles so that:
  - scoresT/ZT chunks come from matmuls lhsT=nq/f[:, nchunk(128)], rhs=xe/x[:, s]
  - exp runs on ACT with accum_out -> den  (free-dim = s reduction)
  - num comes from fused DVE passes: scalar_tensor_tensor(E * ZT, accum=sum_s)
    (TensorTensorReduce crashes the DVE on this silicon; STT's accum works.)

Pipeline: the 8 PSUM banks are the scarce resource. scoresT tiles [128,1152]
(3 banks) are double-buffered (6 banks) so the next image's QK matmuls overlap
the current exp; ZT streams through a 2-slot ring of 1-bank [128,512] tiles,
with the fused multiply split into 3 sub-passes (512/512/128) whose partial
sums land in separate accumulator columns merged at image end. The loop is
chunk-outer/image-inner (weight loads amortize over 4 images) and Z/STT work
is emitted with a 1-image lag so the in-order PE alternates S-bursts with Z
pieces. Inputs are cast to bf16 on the host (halves DMA, enables 1-cyc/row
matmuls); startup DMAs are split and spread across engine queues; per-image
epilogues run pipelined with the last chunk's compute. Z/STT pieces are a
uniform 384 wide: with 512/512/128 pieces every 512-piece waited ~240ns for
its just-in-time Z matmul (ring depth 2 is bank-limited); uniform pieces
equalize the DVE inter-piece window with the Z-matmul chain and put the DVE
at ~96% busy. Measured: 286-288us across runs, rel err 2.87e-3 (bf16
matmul class).

Sharding: batch N=32 across 8 cores (4 images/core), no collectives.
"""

import os
import sys

for _p in ("/opt/trn_rl_repo", "/root/.axon_site/_ro/trn_rl_repo"):
    if os.path.isdir(_p) and _p not in sys.path:
        sys.path.append(_p)

import ml_dtypes
import numpy as np

BF16 = ml_dtypes.bfloat16

N, C, W, H = 32, 128, 32, 36
S = W * H          # 1152
OUT = 5000
CORES = 8
IPC = N // CORES   # images per core = 4
NCH = 40           # n-chunks of 128 partitions (OUT padded to 5120)
OUTP = NCH * 128   # 5120

S_SLICES = [(0, 512), (512, 1024), (1024, 1152)]
# uniform Z/STT pieces: equalizes the DVE inter-piece window with the
# just-in-time Z-matmul chain (512/512/128 made every 512-piece wait ~240ns)
Z_SLICES = [(0, 384), (384, 768), (768, 1152)]

_CACHE = {}


def _build_nc():
    import concourse.tile as tile
    from concourse import bacc, mybir
    from concourse.masks import make_identity

    f32 = mybir.dt.float32
    bf16 = mybir.dt.bfloat16
    EXP = mybir.ActivationFunctionType.Exp
    MULT = mybir.AluOpType.mult
    ADD = mybir.AluOpType.add
    X = mybir.AxisListType.X

    nc = bacc.Bacc()

    x_d = nc.dram_tensor("x", [IPC + 1, C, S], bf16, kind="ExternalInput")
    nq_d = nc.dram_tensor("neuron_query", [C, OUT], bf16, kind="ExternalInput")
    f_d = nc.dram_tensor("features", [C, OUT], bf16, kind="ExternalInput")
    b_d = nc.dram_tensor("bias", [OUT], f32, kind="ExternalInput")
    o_d = nc.dram_tensor("out", [IPC, OUT], f32, kind="ExternalOutput")

    with tile.TileContext(nc) as tc:
        with (
            tc.tile_pool(name="singles", bufs=1) as singles,
            tc.tile_pool(name="imgs", bufs=IPC) as imgs_pool,
            tc.tile_pool(name="accs", bufs=2 * IPC) as acc_pool,
            tc.tile_pool(name="epool", bufs=6) as e_pool,
            tc.tile_pool(name="scpool", bufs=2) as sc_pool,
            tc.tile_pool(name="epi", bufs=2) as epi_pool,
            tc.tile_pool(name="psS", bufs=2, space="PSUM") as psS,
            tc.tile_pool(name="psZ", bufs=2, space="PSUM") as psZ,
        ):
            # ---- one-time loads ----
            # Queue order tuned so chunk 0's deps land first: sync carries
            # pos, x0 and a small first weight piece; gpsimd carries f
            # pieces; scalar carries the remaining images.
            # x0 and pos (packed by the host as x slice IPC) arrive in ONE
            # fused DMA - one less issue+transfer on the startup chain
            x0p_t = singles.tile([C, 2, S], bf16, tag="x0p")
            nc.sync.dma_start(
                out=x0p_t,
                in_=x_d[0 : IPC + 1 : IPC, :, :].rearrange("o p s -> p o s"),
            )
            pos_t = x0p_t[:, 1, :]

            xe_l, xf_l, num3_l, den_l = [], [], [], []
            for i in range(IPC):
                if i == 0:
                    xf_mm = x0p_t[:, 0, :]
                else:
                    xf_mm = imgs_pool.tile([C, S], bf16, tag="xfb")
                    nc.scalar.dma_start(out=xf_mm, in_=x_d[i])
                xe_mm = imgs_pool.tile([C, S], bf16, tag="xe")
                nc.vector.tensor_add(xe_mm, xf_mm, pos_t)  # bf16 2x mode
                xe_l.append(xe_mm)
                xf_l.append(xf_mm)
                num3_t = acc_pool.tile([128, NCH * 3], f32, tag="num3")
                den_t = acc_pool.tile([128, NCH], f32, tag="den")
                num3_l.append(num3_t)
                den_l.append(den_t)

            # weight tiles in pieces (small first piece) so the first
            # chunks' matmuls start after a fraction of the DMA
            PIECES = [256, 1664, 1664, 1536]  # cols per piece, sum=OUTP
            nq_tiles, f_tiles = [], []
            piece_of = []  # chunk -> (piece idx, col offset)
            lo = 0
            for pi, w in enumerate(PIECES):
                hi_real = min(OUT, lo + w)
                nq_p = singles.tile([C, w], bf16, tag=f"nq{pi}")
                f_p = singles.tile([C, w], bf16, tag=f"f{pi}")
                for dram, dst, eng in (
                    (nq_d, nq_p, nc.sync),
                    (f_d, f_p, nc.gpsimd),
                ):
                    if hi_real < lo + w:
                        nc.gpsimd.memset(dst[:, hi_real - lo : w], 0.0)
                    if hi_real > lo:
                        eng.dma_start(
                            out=dst[:, 0 : hi_real - lo], in_=dram[:, lo:hi_real]
                        )
                nq_tiles.append(nq_p)
                f_tiles.append(f_p)
                for c in range(lo // 128, (lo + w) // 128):
                    piece_of.append((pi, c * 128 - lo))
                lo += w

            ident = singles.tile([128, 128], f32)
            make_identity(nc, ident)

            bias_t = singles.tile([128, NCH], f32)
            nc.gpsimd.memset(bias_t, 0.0)
            nc.sync.dma_start(
                out=bias_t[:, 0:39],
                in_=b_d[0 : 39 * 128].rearrange("(c p) -> p c", p=128),
            )
            nc.sync.dma_start(
                out=bias_t[0:8, 39:40],
                in_=b_d[39 * 128 : OUT].rearrange("(p o) -> p o", o=1),
            )

            # ---- main loop: chunk-outer, image-inner ----
            # Z/STT pieces for each image are emitted with a 1-image lag so
            # the in-order PE interleaves S-bursts with Z pieces and each
            # STT's inputs (exp output + Z matmul) are ready when it issues.
            def emit_epilogue(i):
                num_t = epi_pool.tile([128, NCH], f32, tag="num")
                nc.vector.tensor_reduce(
                    out=num_t,
                    in_=num3_l[i].rearrange("p (c j) -> p c j", j=3),
                    op=ADD,
                    axis=X,
                )
                rcp_t = epi_pool.tile([128, NCH], f32, tag="rcp")
                nc.vector.reciprocal(out=rcp_t, in_=den_l[i])
                y1_t = epi_pool.tile([128, NCH], f32, tag="y1")
                nc.vector.tensor_mul(y1_t, num_t, rcp_t)
                y_t = epi_pool.tile([128, NCH], f32, tag="y")
                nc.vector.tensor_add(y_t, y1_t, bias_t)

                pt = psZ.tile([NCH, 128], f32, tag="z")
                nc.tensor.transpose(out=pt, in_=y_t, identity=ident)
                yT_t = epi_pool.tile([NCH, 128], f32, tag="yT")
                nc.scalar.copy(out=yT_t, in_=pt)

                nc.sync.dma_start(
                    out=o_d[i, 0 : 39 * 128].rearrange("(c p) -> c p", p=128),
                    in_=yT_t[0:39, :],
                )
                nc.sync.dma_start(
                    out=o_d[i, 39 * 128 : OUT].rearrange("(o p) -> o p", o=1),
                    in_=yT_t[39:40, 0:8],
                )

            pending = None  # (e_t, img, chunk, f_c)

            def emit_z(e_t, i, c, f_c):
                for j, (lo, hi) in enumerate(Z_SLICES):
                    w = hi - lo
                    z_t = psZ.tile([128, 512], f32, tag="z")
                    nc.tensor.matmul(
                        z_t[:, 0:w], f_c, xf_l[i][:, lo:hi],
                        start=True, stop=True,
                    )
                    sc_t = sc_pool.tile([128, 512], f32, tag="sc")
                    col = c * 3 + j
                    nc.vector.scalar_tensor_tensor(
                        out=sc_t[:, 0:w],
                        in0=e_t[:, lo:hi],
                        scalar=1.0,
                        in1=z_t[:, 0:w],
                        op0=MULT,
                        op1=MULT,
                        accum_out=num3_l[i][:, col : col + 1],
                    )

            for c in range(NCH):
                pi, po = piece_of[c]
                nq_c = nq_tiles[pi][:, po : po + 128]
                f_c = f_tiles[pi][:, po : po + 128]
                for i in range(IPC):
                    s_t = psS.tile([128, S], f32, tag="s")
                    for lo, hi in S_SLICES:
                        nc.tensor.matmul(
                            s_t[:, lo:hi], nq_c, xe_l[i][:, lo:hi],
                            start=True, stop=True,
                        )
                    e_t = e_pool.tile([128, S], f32, tag="e")
                    nc.scalar.activation(
                        out=e_t, in_=s_t, func=EXP,
                        accum_out=den_l[i][:, c : c + 1],
                    )
                    if pending is not None:
                        emit_z(*pending)
                        if pending[2] == NCH - 1:
                            emit_epilogue(pending[1])
                    pending = (e_t, i, c, f_c)
            emit_z(*pending)
            emit_epilogue(pending[1])

    nc.compile()
    return nc


def _get_nc():
    if "nc" not in _CACHE:
        _CACHE["nc"] = _build_nc()
    return _CACHE["nc"]


def _prep_in_maps(inputs):
    xf = np.ascontiguousarray(
        np.asarray(inputs["x"], dtype=np.float32).reshape(N, C, S).astype(BF16)
    )
    nq = np.ascontiguousarray(
        np.asarray(inputs["neuron_query"], dtype=np.float32)
        .reshape(C, OUT)
        .astype(BF16)
    )
    ft = np.ascontiguousarray(
        np.asarray(inputs["features"], dtype=np.float32)
        .reshape(C, OUT)
        .astype(BF16)
    )
    pos = np.ascontiguousarray(
        np.asarray(inputs["pos_emb"], dtype=np.float32).reshape(C, S).astype(BF16)
    )
    bias = np.ascontiguousarray(np.asarray(inputs["bias"], dtype=np.float32))
    return [
        {
            "x": np.ascontiguousarray(
                np.concatenate([xf[i * IPC : (i + 1) * IPC], pos[None]], axis=0)
            ),
            "neuron_query": nq,
            "features": ft,
            "bias": bias,
        }
        for i in range(CORES)
    ]


def run_kernel(inputs, trace=False):
    """Returns (out [N, OUT] float32, BassKernelResults)."""
    from concourse.bass_utils import run_bass_kernel_spmd

    nc = _get_nc()
    in_maps = _prep_in_maps(inputs)
    res = run_bass_kernel_spmd(nc, in_maps, list(range(CORES)), trace=trace)
    out = np.concatenate([r["out"] for r in res.results], axis=0)
    return np.asarray(out, dtype=np.float32), res


def kernel(**inputs):
    out, _ = run_kernel(inputs, trace=False)
    return out
